# revision 6
# baseline (speedup 1.0000x reference)
"""Trainium2 Bass kernel for the 3-layer GATv2 network (nn_GAT_35940286333219).

Sharding: nodes (and their feature rows) are sharded contiguously across the 8
cores (2048 nodes each); edges are partitioned by destination node so the
segment-softmax and scatter-add stay local to the owning core. Each layer
all-gathers the (transformed) source-side features XL = act @ Wl across cores,
then each core processes its local edges grouped into 16 dst-blocks of 128
nodes, with per-edge source rows fetched by indirect DMA from the gathered
table.

Math notes (validated against the reference in fp32):
  - The per-destination softmax max-subtraction is skipped: logits are in
    [-2, 1] for this model family, so exp() is safe and the softmax is
    mathematically identical.
  - att_h . leaky_relu(v) decomposes as 0.6*att.v + 0.4*att.|v|.  The linear
    term separates per-node: sl[n,h] = 0.6*sum_d att[h,d]*(act@Wl)[n,h*D+d]
    which is just act @ Wsl for a host-precomputed [in,H] matrix; likewise sr.
    These ride along as extra columns of the XL / XR matmuls, so per-edge work
    is only: w = xl[src]+xr[dst], |w|, a weighted row-sum, exp, and the
    alpha-weighted one-hot scatter matmul into PSUM.
"""
import os
import numpy as np

import concourse.bacc as bacc
import concourse.bass as bass
import concourse.mybir as mybir
import concourse.tile as tile
from concourse.bass_utils import run_bass_kernel_spmd
from concourse.masks import make_identity

P = 128
N = 16384
NCORES = 8
NLOC = N // NCORES          # 2048
NBLK = NLOC // P            # 16
F_IN = 128
DIM = 64
HID = 256
FP = mybir.dt.float32
I32 = mybir.dt.int32
AF = mybir.ActivationFunctionType
ALU = mybir.AluOpType

LAST_RESULTS = None         # test harness reads exec_time_ns from here


# ----------------------------------------------------------------------------
# Host-side preprocessing
# ----------------------------------------------------------------------------

def _prep_edges(edge_index):
    src = np.concatenate([edge_index[0], np.arange(N, dtype=np.int32)]).astype(np.int64)
    dst = np.concatenate([edge_index[1], np.arange(N, dtype=np.int32)]).astype(np.int64)
    order = np.argsort(dst, kind="stable")
    src_s, dst_s = src[order], dst[order]
    blk = dst_s // P                       # global dst block 0..127
    # counts per (core, block)
    bc = np.bincount(blk, minlength=NCORES * NBLK)
    NT = int(np.ceil(bc.max() / P))
    EBLK = NT * P
    src_pad = np.zeros((NCORES, NBLK, EBLK), dtype=np.int32)
    dst_pad = np.full((NCORES, NBLK, EBLK), P, dtype=np.float32)   # P = pad marker
    starts = np.concatenate([[0], np.cumsum(bc)])
    for g in range(NCORES * NBLK):
        c, b = divmod(g, NBLK)
        s, e = starts[g], starts[g + 1]
        k = e - s
        src_pad[c, b, :k] = src_s[s:e]
        dst_pad[c, b, :k] = (dst_s[s:e] - g * P).astype(np.float32)
    # SBUF layout: [NBLK, P, NT] with [p, t] = edge t*P+p of the block
    src_col = src_pad.reshape(NCORES, NBLK, NT, P).transpose(0, 1, 3, 2).copy()
    dst_col = dst_pad.reshape(NCORES, NBLK, NT, P).transpose(0, 1, 3, 2).copy()
    return src_col, dst_col, NT


def _prep_weights(ii):
    """Build augmented weight matrices.  For each GAT layer:
       WL_ext [inD, C] = [Wl | sl-cols], WR_ext likewise, where
       C = 256 + 1(ones col, H==1 only) + H.
       The 'ones' column supports the denominator accumulation for H==1."""
    out = {}
    for l, H in ((1, 4), (2, 1), (3, 1)):
        Wl = np.asarray(ii[f"Wl{l}"], np.float32)
        Wr = np.asarray(ii[f"Wr{l}"], np.float32)
        att = np.asarray(ii[f"att{l}"], np.float32)          # [H, D]
        inD = Wl.shape[0]
        D = HID // H
        attf = att.reshape(-1)                               # [256]
        # linear part: sl = 0.6 * act @ (Wl * att summed per head)
        Wsl = 0.6 * np.stack([(Wl[:, h * D:(h + 1) * D] * att[h][None, :]).sum(1)
                              for h in range(H)], axis=1)    # [inD, H]
        Wsr = 0.6 * np.stack([(Wr[:, h * D:(h + 1) * D] * att[h][None, :]).sum(1)
                              for h in range(H)], axis=1)
        if H == 1:
            WL = np.concatenate([Wl, np.zeros((inD, 1), np.float32), Wsl], 1)  # [inD,258]
            WR = np.concatenate([Wr, np.zeros((inD, 1), np.float32), Wsr], 1)
        else:
            WL = np.concatenate([Wl, Wsl], 1)                                  # [inD,260]
            WR = np.concatenate([Wr, Wsr], 1)
        out[f"WL{l}"] = WL
        out[f"WR{l}"] = WR
        out[f"svec{l}"] = np.tile(0.4 * attf[None, :], (P, 1)).astype(np.float32)
    return out


# ----------------------------------------------------------------------------
# Bass program
# ----------------------------------------------------------------------------

def _build(NT):
    nc = bacc.Bacc(None)

    def par(name, shape, dtype=FP):
        return nc.declare_dram_parameter(name, list(shape), dtype, isOutput=False)

    # per-core data
    xT = par("xT", [F_IN, NLOC])
    src_col = par("src_col", [NBLK, P, NT], I32)
    dst_col = par("dst_col", [NBLK, P, NT])
    # weights (identical on every core)
    Win = par("Win", [F_IN, DIM]); b_in = par("b_in", [1, DIM])
    Wskip = par("Wskip", [DIM, HID]); bskip = par("bskip", [1, HID])
    WL1 = par("WL1", [DIM, 260]); WR1 = par("WR1", [DIM, 260])
    WL2 = par("WL2", [HID, 258]); WR2 = par("WR2", [HID, 258])
    WL3 = par("WL3", [HID, 258]); WR3 = par("WR3", [HID, 258])
    svec1 = par("svec1", [P, HID]); svec2 = par("svec2", [P, HID]); svec3 = par("svec3", [P, HID])
    Wm1 = par("Wm1", [HID, DIM]); bm1 = par("bm1", [1, DIM])
    Wm2 = par("Wm2", [DIM, DIM]); bm2 = par("bm2", [1, DIM])
    Wm3 = par("Wm3", [DIM, 1]); bm3 = par("bm3", [1, 1])
    iota_f = par("iota_f", [P, P])          # iota_f[p, j] = j
    out = nc.declare_dram_parameter("out", [1, NLOC], FP, isOutput=True)

    # internal DRAM
    CL = {1: 260, 2: 258, 3: 258}
    xl_loc = {l: nc.dram_tensor(f"xl_loc{l}", [NLOC, CL[l]], FP) for l in (1, 2, 3)}
    xl_full = {l: nc.dram_tensor(f"xl_full{l}", [N, CL[l]], FP, addr_space="Shared")
               for l in (1, 2, 3)}

    with tile.TileContext(nc) as tc:
        with (
            tc.tile_pool(name="const", bufs=1) as cp,
            tc.tile_pool(name="big", bufs=1) as bigp,
            tc.tile_pool(name="wk", bufs=1) as wk,
            tc.tile_pool(name="ps_mm", bufs=2, space="PSUM") as ps_mm,
            tc.tile_pool(name="ps_out", bufs=2, space="PSUM") as ps_out_pool,
            tc.tile_pool(name="ps_xr", bufs=2, space="PSUM") as ps_xr_pool,
        ):
            # ---------------- constants to SBUF ----------------
            def load_const(pname, ap, shape, dtype=FP):
                t = cp.tile(list(shape), dtype, name=pname + "_sb")
                nc.sync.dma_start(out=t[:], in_=ap[:])
                return t

            ident = cp.tile([P, P], FP, name="ident")
            make_identity(nc, ident[:])
            ones_row = cp.tile([1, 512], FP, name="ones_row")
            nc.vector.memset(ones_row[:], 1.0)

            def load_const_2k(pname, ap, rows, cols):
                """[2*P, cols] DRAM weight -> [P, 2*cols] SBUF (k-halves side by side)."""
                assert rows == 2 * P
                t = cp.tile([P, 2 * cols], FP, name=pname + "_sb")
                nc.sync.dma_start(out=t[:, :cols], in_=ap[:P, :])
                nc.sync.dma_start(out=t[:, cols:], in_=ap[P:, :])
                return t

            xT_sb = load_const("xT", xT, [F_IN, NLOC])
            Win_sb = load_const("Win", Win, [F_IN, DIM])
            b_in_sb = load_const("b_in", b_in, [1, DIM])
            Wskip_sb = load_const("Wskip", Wskip, [DIM, HID])
            bskip_sb = load_const("bskip", bskip, [1, HID])
            WL_sb = {1: load_const("WL1", WL1, [DIM, 260]),
                     2: load_const_2k("WL2", WL2, HID, 258),
                     3: load_const_2k("WL3", WL3, HID, 258)}
            WR_sb = {1: load_const("WR1", WR1, [DIM, 260]),
                     2: load_const_2k("WR2", WR2, HID, 258),
                     3: load_const_2k("WR3", WR3, HID, 258)}
            svec_sb = {1: load_const("svec1", svec1, [P, HID]),
                       2: load_const("svec2", svec2, [P, HID]),
                       3: load_const("svec3", svec3, [P, HID])}
            Wm1_sb = load_const_2k("Wm1", Wm1, HID, DIM)
            bm1_sb = load_const("bm1", bm1, [1, DIM])
            Wm2_sb = load_const("Wm2", Wm2, [DIM, DIM])
            bm2_sb = load_const("bm2", bm2, [1, DIM])
            Wm3_sb = load_const("Wm3", Wm3, [DIM, 1])
            bm3_sb = load_const("bm3", bm3, [1, 1])
            iof_sb = load_const("iota_f", iota_f, [P, P])

            # persistent activations
            actT = {0: bigp.tile([P, NLOC], FP, name="actT0"),
                    1: bigp.tile([P, NLOC], FP, name="actT1")}
            act_prev = bigp.tile([P, NBLK * HID], FP, name="act_prev")
            act_next = bigp.tile([P, NBLK * HID], FP, name="act_next")
            XRb = bigp.tile([P, NBLK * 260], FP, name="XRb")
            hT = bigp.tile([DIM, NLOC], FP, name="hT")
            m1T = bigp.tile([DIM, NLOC], FP, name="m1T")
            m2T = bigp.tile([DIM, NLOC], FP, name="m2T")
            y_sb = bigp.tile([1, NLOC], FP, name="y_sb")

            # ---------------- phase A: hT = relu(Win.T @ x + b_in) -----------
            for j in range(NLOC // 512):
                sl = slice(j * 512, (j + 1) * 512)
                pmm = ps_mm.tile([P, 512], FP, space="PSUM", name="pmm", tag="pmm")
                nc.tensor.matmul(out=pmm[:DIM, :], lhsT=Win_sb[:], rhs=xT_sb[:, sl],
                                 start=True, stop=False)
                nc.tensor.matmul(out=pmm[:DIM, :], lhsT=b_in_sb[:], rhs=ones_row[:],
                                 start=False, stop=True)
                nc.scalar.activation(out=hT[:DIM, sl], in_=pmm[:DIM, :], func=AF.Relu)

            # XL1/XR1 (node-major) and skip1 -> act_prev
            C = CL[1]
            for b in range(NBLK):
                nsl = slice(b * P, (b + 1) * P)
                pxl = ps_mm.tile([P, C], FP, space="PSUM", name="pxl", tag="pmm")
                nc.tensor.matmul(out=pxl[:], lhsT=hT[:DIM, nsl], rhs=WL_sb[1][:],
                                 start=True, stop=True)
                xl_t = wk.tile([P, C], FP, name="xl_st", tag="xl_st", bufs=3)
                nc.scalar.activation(out=xl_t[:], in_=pxl[:], func=AF.Copy)
                nc.sync.dma_start(out=xl_loc[1][nsl, :], in_=xl_t[:])

                pxr = ps_mm.tile([P, C], FP, space="PSUM", name="pxr", tag="pmm")
                nc.tensor.matmul(out=pxr[:], lhsT=hT[:DIM, nsl], rhs=WR_sb[1][:],
                                 start=True, stop=True)
                nc.scalar.activation(out=XRb[:, b * 260:b * 260 + C], in_=pxr[:], func=AF.Copy)

                psk = ps_mm.tile([P, HID], FP, space="PSUM", name="psk", tag="pmm")
                nc.tensor.matmul(out=psk[:], lhsT=hT[:DIM, nsl], rhs=Wskip_sb[:],
                                 start=True, stop=False)
                nc.tensor.matmul(out=psk[:], lhsT=ones_row[:, :P], rhs=bskip_sb[:],
                                 start=False, stop=True)
                nc.scalar.activation(out=act_prev[:, b * HID:(b + 1) * HID], in_=psk[:],
                                     func=AF.Copy)

            nc.gpsimd.collective_compute(
                "AllGather", ALU.bypass, replica_groups=[list(range(NCORES))],
                ins=[xl_loc[1][:]], outs=[xl_full[1][:]])

            # ---------------- edge stage + next-layer prep, per layer --------
            def edge_layer(l, H):
                """Consumes xl_full[l], XRb, act_prev; writes act_next and actT."""
                C = CL[l]
                D = HID // H
                for b in range(NBLK):
                    src_b = wk.tile([P, NT], I32, name="src_b", tag="src_b", bufs=2)
                    nc.sync.dma_start(out=src_b[:], in_=src_col[b])
                    dst_b = wk.tile([P, NT], FP, name="dst_b", tag="dst_b", bufs=2)
                    nc.sync.dma_start(out=dst_b[:], in_=dst_col[b])
                    ps_o = ps_out_pool.tile([P, 264], FP, space="PSUM",
                                            name="ps_o", tag="ps_o")
                    for t in range(NT):
                        xl_t = wk.tile([P, C], FP, name="xl_t", tag="xl_t", bufs=8)
                        nc.gpsimd.indirect_dma_start(
                            out=xl_t[:], out_offset=None, in_=xl_full[l][:],
                            in_offset=bass.IndirectOffsetOnAxis(ap=src_b[:, t:t + 1], axis=0))
                        # one-hot M [edge, dstlocal] and its transpose
                        m_t = wk.tile([P, P], FP, name="m_t", tag="m_t", bufs=4)
                        nc.vector.tensor_tensor(
                            out=m_t[:], in0=dst_b[:, t:t + 1].to_broadcast([P, P]),
                            in1=iof_sb[:], op=ALU.is_equal)
                        mt_ps = ps_xr_pool.tile([P, P], FP, space="PSUM",
                                                name="mt_ps", tag="mt_ps")
                        nc.tensor.transpose(out=mt_ps[:], in_=m_t[:], identity=ident[:])
                        mt_t = wk.tile([P, P], FP, name="mt_t", tag="mt_t", bufs=4)
                        nc.scalar.activation(out=mt_t[:], in_=mt_ps[:], func=AF.Copy)
                        # xr-expand: [edge, C] = M @ XR_block
                        xr_ps = ps_xr_pool.tile([P, 264], FP, space="PSUM",
                                                name="xr_ps", tag="xr_ps")
                        nc.tensor.matmul(out=xr_ps[:, :C], lhsT=mt_t[:],
                                         rhs=XRb[:, b * 260:b * 260 + C],
                                         start=True, stop=True)
                        # w = xl + xr   (last H cols become sl+sr)
                        w_t = wk.tile([P, C], FP, name="w_t", tag="w_t", bufs=4)
                        nc.vector.tensor_tensor(out=w_t[:], in0=xl_t[:], in1=xr_ps[:, :C],
                                                op=ALU.add)
                        # |w| then z = svec * |w|, r_h = sum_d z
                        a_t = wk.tile([P, HID], FP, name="a_t", tag="a_t", bufs=4)
                        nc.scalar.activation(out=a_t[:], in_=w_t[:, :HID], func=AF.Abs)
                        z_t = wk.tile([P, HID], FP, name="z_t", tag="z_t", bufs=4)
                        nc.vector.tensor_tensor(out=z_t[:], in0=a_t[:], in1=svec_sb[l][:],
                                                op=ALU.mult)
                        r_t = wk.tile([P, H], FP, name="r_t", tag="r_t", bufs=4)
                        nc.vector.tensor_reduce(
                            out=r_t[:], in_=z_t[:].rearrange("p (h d) -> p h d", h=H),
                            axis=mybir.AxisListType.X, op=ALU.add)
                        if H == 1:
                            # alpha = exp(r + (sl+sr));  M' = M * alpha
                            alpha = wk.tile([P, 1], FP, name="alpha", tag="alpha", bufs=4)
                            nc.scalar.activation(out=alpha[:], in_=r_t[:], func=AF.Exp,
                                                 bias=w_t[:, 257:258])
                            mp_t = wk.tile([P, P], FP, name="mp_t", tag="mp_t", bufs=4)
                            nc.vector.tensor_scalar(out=mp_t[:], in0=m_t[:],
                                                    scalar1=alpha[:, :1], scalar2=None,
                                                    op0=ALU.mult)
                            nc.tensor.matmul(out=ps_o[:, :257], lhsT=mp_t[:],
                                             rhs=xl_t[:, :257],
                                             start=(t == 0), stop=(t == NT - 1))
                        else:
                            lg = wk.tile([P, H], FP, name="lg", tag="lg", bufs=4)
                            nc.vector.tensor_tensor(out=lg[:], in0=r_t[:],
                                                    in1=w_t[:, HID:HID + H], op=ALU.add)
                            v_t = wk.tile([P, HID + H], FP, name="v_t", tag="v_t", bufs=4)
                            nc.scalar.activation(out=v_t[:, HID:HID + H], in_=lg[:],
                                                 func=AF.Exp)
                            for h in range(H):
                                nc.vector.tensor_scalar(
                                    out=v_t[:, h * D:(h + 1) * D],
                                    in0=xl_t[:, h * D:(h + 1) * D],
                                    scalar1=v_t[:, HID + h:HID + h + 1], scalar2=None,
                                    op0=ALU.mult)
                            nc.tensor.matmul(out=ps_o[:, :HID + H], lhsT=m_t[:],
                                             rhs=v_t[:],
                                             start=(t == 0), stop=(t == NT - 1))
                    # -------- block finalize: divide, relu, residual ---------
                    dcol = HID if H == 1 else HID  # denom cols start
                    nden = 1 if H == 1 else H
                    rec = wk.tile([P, nden], FP, name="rec", tag="rec", bufs=2)
                    nc.vector.reciprocal(out=rec[:], in_=ps_o[:, dcol:dcol + nden])
                    g_t = wk.tile([P, HID], FP, name="g_t", tag="g_t", bufs=2)
                    for h in range(nden):
                        hs = slice(h * (HID // nden), (h + 1) * (HID // nden))
                        nc.vector.tensor_scalar(out=g_t[:, hs], in0=ps_o[:, hs],
                                                scalar1=rec[:, h:h + 1], scalar2=None,
                                                op0=ALU.mult)
                    gr = wk.tile([P, HID], FP, name="gr", tag="gr", bufs=2)
                    nc.scalar.activation(out=gr[:], in_=g_t[:], func=AF.Relu)
                    bsl = slice(b * HID, (b + 1) * HID)
                    nc.vector.tensor_tensor(out=act_next[:, bsl], in0=gr[:],
                                            in1=act_prev[:, bsl], op=ALU.add)
                    # transposes for next-layer matmuls
                    for k in range(2):
                        tp = ps_mm.tile([P, P], FP, space="PSUM", name="tp", tag="pmm")
                        nc.tensor.transpose(out=tp[:], in_=act_next[:, b * HID + k * P:
                                                                    b * HID + (k + 1) * P],
                                            identity=ident[:])
                        nc.scalar.activation(out=actT[k][:, b * P:(b + 1) * P],
                                             in_=tp[:], func=AF.Copy)

            def xlxr_layer(l):
                """act_next/actT -> xl_loc[l] (+AllGather) and XRb."""
                C = CL[l]
                for b in range(NBLK):
                    nsl = slice(b * P, (b + 1) * P)
                    pxl = ps_mm.tile([P, C], FP, space="PSUM", name="pxl2", tag="pmm")
                    for k in range(2):
                        nc.tensor.matmul(out=pxl[:], lhsT=actT[k][:, nsl],
                                         rhs=WL_sb[l][:, k * C:(k + 1) * C],
                                         start=(k == 0), stop=False)
                    nc.tensor.matmul(out=pxl[:], lhsT=ones_row[:, :P],
                                     rhs=_ones_col_row(l), start=False, stop=True)
                    xl_t = wk.tile([P, C], FP, name="xl_st2", tag="xl_st", bufs=3)
                    nc.scalar.activation(out=xl_t[:], in_=pxl[:], func=AF.Copy)
                    nc.sync.dma_start(out=xl_loc[l][nsl, :], in_=xl_t[:])

                    pxr = ps_mm.tile([P, C], FP, space="PSUM", name="pxr2", tag="pmm")
                    for k in range(2):
                        nc.tensor.matmul(out=pxr[:], lhsT=actT[k][:, nsl],
                                         rhs=WR_sb[l][:, k * C:(k + 1) * C],
                                         start=(k == 0), stop=(k == 1))
                    nc.scalar.activation(out=XRb[:, b * 260:b * 260 + C], in_=pxr[:],
                                         func=AF.Copy)
                nc.gpsimd.collective_compute(
                    "AllGather", ALU.bypass, replica_groups=[list(range(NCORES))],
                    ins=[xl_loc[l][:]], outs=[xl_full[l][:]])

            # ones column (col 256) for H=1 layers, via a K=1 matmul row
            _ocr = {}

            def _ones_col_row(l):
                if l not in _ocr:
                    t = cp.tile([1, CL[l]], FP, name=f"onescol{l}")
                    nc.vector.memset(t[:], 0.0)
                    nc.vector.memset(t[:, HID:HID + 1], 1.0)
                    _ocr[l] = t
                return _ocr[l][:]

            # layer 1
            edge_layer(1, 4)
            # swap act buffers: act_next of layer1 is act input/residual of layer2
            act_prev, act_next = act_next, act_prev
            xlxr_layer(2)
            edge_layer(2, 1)
            act_prev, act_next = act_next, act_prev
            xlxr_layer(3)
            edge_layer(3, 1)
            act_prev, act_next = act_next, act_prev
            # act_prev now holds out3 (node-major) and actT holds its transpose

            # ---------------- MLP head ----------------
            for j in range(NLOC // 512):
                sl = slice(j * 512, (j + 1) * 512)
                pm1 = ps_mm.tile([P, 512], FP, space="PSUM", name="pm1", tag="pmm")
                for k in range(2):
                    nc.tensor.matmul(out=pm1[:DIM, :], lhsT=Wm1_sb[:, k * DIM:(k + 1) * DIM],
                                     rhs=actT[k][:, sl], start=(k == 0), stop=False)
                nc.tensor.matmul(out=pm1[:DIM, :], lhsT=bm1_sb[:], rhs=ones_row[:],
                                 start=False, stop=True)
                nc.scalar.activation(out=m1T[:DIM, sl], in_=pm1[:DIM, :], func=AF.Relu)
            for j in range(NLOC // 512):
                sl = slice(j * 512, (j + 1) * 512)
                pm2 = ps_mm.tile([P, 512], FP, space="PSUM", name="pm2", tag="pmm")
                nc.tensor.matmul(out=pm2[:DIM, :], lhsT=Wm2_sb[:], rhs=m1T[:DIM, sl],
                                 start=True, stop=False)
                nc.tensor.matmul(out=pm2[:DIM, :], lhsT=bm2_sb[:], rhs=ones_row[:],
                                 start=False, stop=True)
                nc.scalar.activation(out=m2T[:DIM, sl], in_=pm2[:DIM, :], func=AF.Relu)
            for j in range(NLOC // 512):
                sl = slice(j * 512, (j + 1) * 512)
                py = ps_mm.tile([P, 512], FP, space="PSUM", name="py", tag="pmm")
                nc.tensor.matmul(out=py[:1, :], lhsT=Wm3_sb[:], rhs=m2T[:DIM, sl],
                                 start=True, stop=False)
                nc.tensor.matmul(out=py[:1, :], lhsT=bm3_sb[:], rhs=ones_row[:],
                                 start=False, stop=True)
                nc.scalar.activation(out=y_sb[:, sl], in_=py[:1, :], func=AF.Copy)
            nc.sync.dma_start(out=out[:], in_=y_sb[:])

    nc.compile()
    return nc


_BUILD_CACHE = {}


def _get_program(NT):
    if NT not in _BUILD_CACHE:
        _BUILD_CACHE[NT] = _build(NT)
    return _BUILD_CACHE[NT]


# ----------------------------------------------------------------------------
# Entry point
# ----------------------------------------------------------------------------

def kernel(**inputs) -> np.ndarray:
    global LAST_RESULTS
    ii = {k: np.asarray(v) for k, v in inputs.items()}
    assert ii["x"].shape == (N, F_IN)
    for l in (1, 2, 3):
        assert not np.any(ii[f"b{l}"]), "GAT bias assumed zero"

    src_col, dst_col, NT = _prep_edges(np.asarray(ii["edge_index"], np.int64))
    w = _prep_weights(ii)
    iota = np.arange(P, dtype=np.float32)
    iota_f = np.tile(iota[None, :], (P, 1))

    common = dict(
        Win=np.asarray(ii["Win"], np.float32),
        b_in=np.asarray(ii["b_in"], np.float32)[None, :],
        Wskip=np.asarray(ii["Wskip"], np.float32),
        bskip=np.asarray(ii["bskip"], np.float32)[None, :],
        WL1=w["WL1"], WR1=w["WR1"], svec1=w["svec1"],
        WL2=w["WL2"], WR2=w["WR2"], svec2=w["svec2"],
        WL3=w["WL3"], WR3=w["WR3"], svec3=w["svec3"],
        Wm1=np.asarray(ii["Wm1"], np.float32),
        bm1=np.asarray(ii["bm1"], np.float32)[None, :],
        Wm2=np.asarray(ii["Wm2"], np.float32),
        bm2=np.asarray(ii["bm2"], np.float32)[None, :],
        Wm3=np.asarray(ii["Wm3"], np.float32),
        bm3=np.asarray(ii["bm3"], np.float32)[None, :],
        iota_f=iota_f,
    )
    x = np.asarray(ii["x"], np.float32)
    in_maps = []
    for c in range(NCORES):
        m = dict(common)
        m["xT"] = np.ascontiguousarray(x[c * NLOC:(c + 1) * NLOC].T)
        m["src_col"] = src_col[c]
        m["dst_col"] = dst_col[c]
        in_maps.append(m)

    nc = _get_program(NT)
    res = run_bass_kernel_spmd(nc, in_maps, list(range(NCORES)),
                               trace=bool(os.environ.get("GAT_TRACE")))
    LAST_RESULTS = res
    return np.concatenate([res.results[c]["out"].reshape(-1) for c in range(NCORES)])


# revision 14
# speedup vs baseline: 1.3877x; 1.3877x over previous
"""Trainium2 Bass kernel for the 3-layer GATv2 network (nn_GAT_35940286333219).

Sharding: nodes contiguously across 8 cores (2048 each); edges partitioned by
destination so segment-softmax/scatter-add stay local; per-layer AllGather of
the source-side transformed features XL = act @ Wl; per-edge source rows via
indirect DMA gather from the gathered table.

v2: bf16 compute (PSUM accumulation fp32), block-batched one-hot builds and
exp, w = xl[src]+xr[dst] computed on the TensorEngine via a paired
(M_T @ XR + I @ xl) PSUM accumulation, per-edge logits via one fused
tensor_tensor_reduce per head with the separable linear term riding as extra
XL/XR columns (att.lrelu(v) = 0.6*att.v + 0.4*att.|v|).
"""
import os
import numpy as np
import ml_dtypes

import concourse.bacc as bacc
import concourse.bass as bass
import concourse.mybir as mybir
import concourse.tile as tile
from concourse.bass_utils import run_bass_kernel_spmd
from concourse.masks import make_identity

P = 128
N = 16384
NCORES = 8
NLOC = N // NCORES          # 2048
NBLK = NLOC // P            # 16
F_IN = 128
DIM = 64
HID = 256
FP = mybir.dt.float32
BF = mybir.dt.bfloat16
I32 = mybir.dt.int32
AF = mybir.ActivationFunctionType
ALU = mybir.AluOpType
BF_NP = ml_dtypes.bfloat16

LAST_RESULTS = None


def _prep_edges(edge_index):
    src = np.concatenate([edge_index[0], np.arange(N, dtype=np.int32)]).astype(np.int64)
    dst = np.concatenate([edge_index[1], np.arange(N, dtype=np.int32)]).astype(np.int64)
    order = np.argsort(dst, kind="stable")
    src_s, dst_s = src[order], dst[order]
    blk = dst_s // P
    bc = np.bincount(blk, minlength=NCORES * NBLK)
    NT = int(np.ceil(bc.max() / P))
    EBLK = NT * P
    src_pad = np.zeros((NCORES, NBLK, EBLK), dtype=np.int32)
    dst_pad = np.full((NCORES, NBLK, EBLK), P, dtype=np.float32)   # P = pad marker
    starts = np.concatenate([[0], np.cumsum(bc)])
    for g in range(NCORES * NBLK):
        c, b = divmod(g, NBLK)
        s, e = starts[g], starts[g + 1]
        k = e - s
        src_pad[c, b, :k] = src_s[s:e]
        dst_pad[c, b, :k] = (dst_s[s:e] - g * P).astype(np.float32)
    src_col = src_pad.reshape(NCORES, NBLK, NT, P).transpose(0, 1, 3, 2).copy()
    dst_col = dst_pad.reshape(NCORES, NBLK, NT, P).transpose(0, 1, 3, 2).copy()
    return src_col, dst_col.astype(BF_NP), NT


def _prep_weights(ii):
    out = {}
    for l, H in ((1, 4), (2, 1), (3, 1)):
        Wl = np.asarray(ii[f"Wl{l}"], np.float32)
        Wr = np.asarray(ii[f"Wr{l}"], np.float32)
        att = np.asarray(ii[f"att{l}"], np.float32)
        inD = Wl.shape[0]
        D = HID // H
        attf = att.reshape(-1)
        Wsl = 0.6 * np.stack([(Wl[:, h * D:(h + 1) * D] * att[h][None, :]).sum(1)
                              for h in range(H)], axis=1)
        Wsr = 0.6 * np.stack([(Wr[:, h * D:(h + 1) * D] * att[h][None, :]).sum(1)
                              for h in range(H)], axis=1)
        if H == 1:
            WL = np.concatenate([Wl, np.zeros((inD, 1), np.float32), Wsl], 1)
            WR = np.concatenate([Wr, np.zeros((inD, 1), np.float32), Wsr], 1)
        else:
            WL = np.concatenate([Wl, Wsl], 1)
            WR = np.concatenate([Wr, Wsr], 1)
        out[f"WL{l}"] = WL.astype(BF_NP)
        out[f"WR{l}"] = WR.astype(BF_NP)
        out[f"svec{l}"] = np.tile(0.4 * attf[None, :], (P, 1)).astype(BF_NP)
    return out


def _build(NT):
    nc = bacc.Bacc(None)

    def par(name, shape, dtype=BF):
        return nc.declare_dram_parameter(name, list(shape), dtype, isOutput=False)

    xT = par("xT", [F_IN, NLOC])
    src_col = par("src_col", [NBLK, P, NT], I32)
    dst_col = par("dst_col", [NBLK, P, NT], BF)
    Win = par("Win", [F_IN, DIM]); b_in = par("b_in", [1, DIM])
    Wskip = par("Wskip", [DIM, HID]); bskip = par("bskip", [1, HID])
    WL1 = par("WL1", [DIM, 260]); WR1 = par("WR1", [DIM, 260])
    WL2 = par("WL2", [HID, 258]); WR2 = par("WR2", [HID, 258])
    WL3 = par("WL3", [HID, 258]); WR3 = par("WR3", [HID, 258])
    svec1 = par("svec1", [P, HID]); svec2 = par("svec2", [P, HID]); svec3 = par("svec3", [P, HID])
    Wm1 = par("Wm1", [HID, DIM]); bm1 = par("bm1", [1, DIM])
    Wm2 = par("Wm2", [DIM, DIM]); bm2 = par("bm2", [1, DIM])
    Wm3 = par("Wm3", [DIM, 1]); bm3 = par("bm3", [1, 1])
    iota_f = par("iota_f", [P, P])
    out = nc.declare_dram_parameter("out", [1, NLOC], FP, isOutput=True)

    CL = {1: 260, 2: 258, 3: 258}
    xl_loc = {l: nc.dram_tensor(f"xl_loc{l}", [NLOC, CL[l]], BF) for l in (1, 2, 3)}
    xl_full = {l: nc.dram_tensor(f"xl_full{l}", [N, CL[l]], BF, addr_space="Shared")
               for l in (1, 2, 3)}

    with tile.TileContext(nc) as tc:
        with (
            tc.tile_pool(name="const", bufs=1) as cp,
            tc.tile_pool(name="big", bufs=1) as bigp,
            tc.tile_pool(name="wk", bufs=1) as wk,
            tc.tile_pool(name="ps_mm", bufs=2, space="PSUM") as ps_mm,
            tc.tile_pool(name="ps_out", bufs=2, space="PSUM") as ps_out_pool,
            tc.tile_pool(name="ps_w", bufs=2, space="PSUM") as ps_w_pool,
        ):
            def load_const(pname, ap, shape, dtype=BF):
                t = cp.tile(list(shape), dtype, name=pname + "_sb")
                nc.sync.dma_start(out=t[:], in_=ap[:])
                return t

            def load_const_2k(pname, ap, rows, cols):
                assert rows == 2 * P
                t = cp.tile([P, 2 * cols], BF, name=pname + "_sb")
                nc.sync.dma_start(out=t[:, :cols], in_=ap[:P, :])
                nc.sync.dma_start(out=t[:, cols:], in_=ap[P:, :])
                return t

            ident_f = cp.tile([P, P], FP, name="ident_f")
            make_identity(nc, ident_f[:])
            ident_b = cp.tile([P, P], BF, name="ident_b")
            nc.vector.tensor_copy(out=ident_b[:], in_=ident_f[:])
            ones_row = cp.tile([1, 512], BF, name="ones_row")
            nc.vector.memset(ones_row[:], 1.0)

            xT_sb = load_const("xT", xT, [F_IN, NLOC])
            Win_sb = load_const("Win", Win, [F_IN, DIM])
            b_in_sb = load_const("b_in", b_in, [1, DIM])
            Wskip_sb = load_const("Wskip", Wskip, [DIM, HID])
            bskip_sb = load_const("bskip", bskip, [1, HID])
            WL_sb = {1: load_const("WL1", WL1, [DIM, 260]),
                     2: load_const_2k("WL2", WL2, HID, 258),
                     3: load_const_2k("WL3", WL3, HID, 258)}
            WR_sb = {1: load_const("WR1", WR1, [DIM, 260]),
                     2: load_const_2k("WR2", WR2, HID, 258),
                     3: load_const_2k("WR3", WR3, HID, 258)}
            svec_sb = {1: load_const("svec1", svec1, [P, HID]),
                       2: load_const("svec2", svec2, [P, HID]),
                       3: load_const("svec3", svec3, [P, HID])}
            Wm1_sb = load_const_2k("Wm1", Wm1, HID, DIM)
            bm1_sb = load_const("bm1", bm1, [1, DIM])
            Wm2_sb = load_const("Wm2", Wm2, [DIM, DIM])
            bm2_sb = load_const("bm2", bm2, [1, DIM])
            Wm3_sb = load_const("Wm3", Wm3, [DIM, 1])
            bm3_sb = load_const("bm3", bm3, [1, 1])
            iof_sb = load_const("iota_f", iota_f, [P, P])

            actT = {0: bigp.tile([P, NLOC], BF, name="actT0"),
                    1: bigp.tile([P, NLOC], BF, name="actT1")}
            act_prev = bigp.tile([P, NBLK * HID], FP, name="act_prev")
            act_next = bigp.tile([P, NBLK * HID], FP, name="act_next")
            XRb = bigp.tile([P, NBLK * 260], BF, name="XRb")
            hT = bigp.tile([DIM, NLOC], BF, name="hT")
            m1T = bigp.tile([DIM, NLOC], BF, name="m1T")
            m2T = bigp.tile([DIM, NLOC], BF, name="m2T")
            y_sb = bigp.tile([1, NLOC], FP, name="y_sb")

            # ---------------- phase A ----------------
            for j in range(NLOC // 512):
                sl = slice(j * 512, (j + 1) * 512)
                pmm = ps_mm.tile([P, 512], FP, space="PSUM", name="pmm", tag="pmm")
                nc.tensor.matmul(out=pmm[:DIM, :], lhsT=Win_sb[:], rhs=xT_sb[:, sl],
                                 start=True, stop=False)
                nc.tensor.matmul(out=pmm[:DIM, :], lhsT=b_in_sb[:], rhs=ones_row[:],
                                 start=False, stop=True)
                nc.scalar.activation(out=hT[:DIM, sl], in_=pmm[:DIM, :], func=AF.Relu)

            C1 = CL[1]
            for b in range(NBLK):
                nsl = slice(b * P, (b + 1) * P)
                pxl = ps_mm.tile([P, C1], FP, space="PSUM", name="pxl", tag="pmm")
                nc.tensor.matmul(out=pxl[:], lhsT=hT[:DIM, nsl], rhs=WL_sb[1][:],
                                 start=True, stop=True)
                xl_st = wk.tile([P, C1], BF, name="xl_st", tag="xl_st", bufs=3)
                nc.scalar.activation(out=xl_st[:], in_=pxl[:], func=AF.Copy)
                nc.sync.dma_start(out=xl_loc[1][nsl, :], in_=xl_st[:])

                pxr = ps_mm.tile([P, C1], FP, space="PSUM", name="pxr", tag="pmm")
                nc.tensor.matmul(out=pxr[:], lhsT=hT[:DIM, nsl], rhs=WR_sb[1][:],
                                 start=True, stop=True)
                nc.scalar.activation(out=XRb[:, b * 260:b * 260 + C1], in_=pxr[:], func=AF.Copy)

                psk = ps_mm.tile([P, HID], FP, space="PSUM", name="psk", tag="pmm")
                nc.tensor.matmul(out=psk[:], lhsT=hT[:DIM, nsl], rhs=Wskip_sb[:],
                                 start=True, stop=False)
                nc.tensor.matmul(out=psk[:], lhsT=ones_row[:, :P], rhs=bskip_sb[:],
                                 start=False, stop=True)
                nc.scalar.activation(out=act_prev[:, b * HID:(b + 1) * HID], in_=psk[:],
                                     func=AF.Copy)

            nc.gpsimd.collective_compute(
                "AllGather", ALU.bypass, replica_groups=[list(range(NCORES))],
                ins=[xl_loc[1][:]], outs=[xl_full[1][:]])

            # ---------------- edge stage ----------------
            def edge_layer(l, H):
                C = CL[l]
                D = HID // H
                for b in range(NBLK):
                    src_b = wk.tile([P, NT], I32, name="src_b", tag="src_b", bufs=2)
                    nc.sync.dma_start(out=src_b[:], in_=src_col[b])
                    dst_b = wk.tile([P, NT], BF, name="dst_b", tag="dst_b", bufs=2)
                    nc.sync.dma_start(out=dst_b[:], in_=dst_col[b])
                    m_all = wk.tile([P, NT * P], BF, name="m_all", tag="m_all", bufs=2)
                    nc.vector.tensor_tensor(
                        out=m_all[:].rearrange("p (t e) -> p t e", t=NT),
                        in0=dst_b[:, :, None].to_broadcast([P, NT, P]),
                        in1=iof_sb[:, None, :].to_broadcast([P, NT, P]),
                        op=ALU.is_equal)
                    xl_all = wk.tile([P, NT * C], BF, name="xl_all", tag="xl_all", bufs=2)
                    for t in range(NT):
                        nc.gpsimd.indirect_dma_start(
                            out=xl_all[:, t * C:(t + 1) * C], out_offset=None,
                            in_=xl_full[l][:],
                            in_offset=bass.IndirectOffsetOnAxis(ap=src_b[:, t:t + 1], axis=0))
                    r_all = wk.tile([P, NT * H], FP, name="r_all", tag="r_all", bufs=2)
                    alpha_all = wk.tile([P, NT * H], FP, name="alpha_all",
                                        tag="alpha_all", bufs=2)
                    ps_o = ps_out_pool.tile([P, 264], FP, space="PSUM",
                                            name="ps_o", tag="ps_o")
                    for t in range(NT):
                        xl_t = xl_all[:, t * C:(t + 1) * C]
                        m_t = m_all[:, t * P:(t + 1) * P]
                        mt_ps = ps_w_pool.tile([P, P], BF, space="PSUM",
                                               name="mt_ps", tag="mt_ps")
                        nc.tensor.transpose(out=mt_ps[:], in_=m_t, identity=ident_b[:])
                        mt_t = wk.tile([P, P], BF, name="mt_t", tag="mt_t", bufs=4)
                        nc.scalar.activation(out=mt_t[:], in_=mt_ps[:], func=AF.Copy)
                        # w = M @ XR_blk + xl   (PE does the add via PSUM accum)
                        w_ps = ps_w_pool.tile([P, 264], FP, space="PSUM",
                                              name="w_ps", tag="w_ps")
                        nc.tensor.matmul(out=w_ps[:, :C], lhsT=mt_t[:],
                                         rhs=XRb[:, b * 260:b * 260 + C],
                                         start=True, stop=False)
                        nc.tensor.matmul(out=w_ps[:, :C], lhsT=ident_b[:],
                                         rhs=xl_t, start=False, stop=True)
                        a_t = wk.tile([P, HID], BF, name="a_t", tag="a_t", bufs=4)
                        nc.scalar.activation(out=a_t[:], in_=w_ps[:, :HID], func=AF.Abs)
                        off0 = HID + 1 if H == 1 else HID
                        slsr = wk.tile([P, 4], FP, name="slsr", tag="slsr", bufs=4)
                        nc.vector.tensor_scalar(out=slsr[:, :H], in0=w_ps[:, off0:off0 + H],
                                                scalar1=1.0, scalar2=None, op0=ALU.mult)
                        z_t = wk.tile([P, HID], BF, name="z_t", tag="z_t", bufs=2)
                        nc.vector.tensor_tensor(out=z_t[:], in0=a_t[:], in1=svec_sb[l][:],
                                                op=ALU.mult)
                        rr = wk.tile([P, 4], FP, name="rr", tag="rr", bufs=4)
                        nc.vector.tensor_reduce(
                            out=rr[:, :H], in_=z_t[:].rearrange("p (h d) -> p h d", h=H),
                            axis=mybir.AxisListType.X, op=ALU.add)
                        nc.vector.tensor_tensor(out=r_all[:, t * H:(t + 1) * H],
                                                in0=rr[:, :H], in1=slsr[:, :H],
                                                op=ALU.add)
                    nc.scalar.activation(out=alpha_all[:], in_=r_all[:], func=AF.Exp)
                    for t in range(NT):
                        xl_t = xl_all[:, t * C:(t + 1) * C]
                        if H == 1:
                            mp_t = wk.tile([P, P], BF, name="mp_t", tag="mp_t", bufs=4)
                            nc.vector.tensor_scalar(
                                out=mp_t[:], in0=m_all[:, t * P:(t + 1) * P],
                                scalar1=alpha_all[:, t:t + 1], scalar2=None,
                                op0=ALU.mult)
                            nc.tensor.matmul(out=ps_o[:, :257], lhsT=mp_t[:],
                                             rhs=xl_all[:, t * C:t * C + 257],
                                             start=(t == 0), stop=(t == NT - 1))
                        else:
                            v_t = wk.tile([P, HID + 4], BF, name="v_t", tag="v_t", bufs=4)
                            for h in range(H):
                                nc.vector.tensor_scalar(
                                    out=v_t[:, h * D:(h + 1) * D],
                                    in0=xl_all[:, t * C + h * D:t * C + (h + 1) * D],
                                    scalar1=alpha_all[:, t * H + h:t * H + h + 1],
                                    scalar2=None, op0=ALU.mult)
                            nc.vector.tensor_copy(
                                out=v_t[:, HID:HID + H],
                                in_=alpha_all[:, t * H:(t + 1) * H])
                            nc.tensor.matmul(out=ps_o[:, :HID + H], lhsT=m_all[:, t * P:(t + 1) * P],
                                             rhs=v_t[:], start=(t == 0), stop=(t == NT - 1))
                    # finalize
                    rec = wk.tile([P, H], FP, name="rec", tag="rec", bufs=2)
                    nc.vector.reciprocal(out=rec[:], in_=ps_o[:, HID:HID + H])
                    g_t = wk.tile([P, HID], FP, name="g_t", tag="g_t", bufs=2)
                    for h in range(H):
                        hs = slice(h * D, (h + 1) * D)
                        nc.vector.tensor_scalar(out=g_t[:, hs], in0=ps_o[:, hs],
                                                scalar1=rec[:, h:h + 1], scalar2=None,
                                                op0=ALU.mult)
                    gr = wk.tile([P, HID], FP, name="gr", tag="gr", bufs=2)
                    nc.scalar.activation(out=gr[:], in_=g_t[:], func=AF.Relu)
                    bsl = slice(b * HID, (b + 1) * HID)
                    nc.vector.tensor_tensor(out=act_next[:, bsl], in0=gr[:],
                                            in1=act_prev[:, bsl], op=ALU.add)
                    for k in range(2):
                        tp = ps_mm.tile([P, P], FP, space="PSUM", name="tp", tag="pmm")
                        nc.tensor.transpose(
                            out=tp[:], in_=act_next[:, b * HID + k * P:b * HID + (k + 1) * P],
                            identity=ident_f[:])
                        nc.scalar.activation(out=actT[k][:, b * P:(b + 1) * P],
                                             in_=tp[:], func=AF.Copy)

            _ocr = {}

            def _ones_col_row(l):
                if l not in _ocr:
                    t = cp.tile([1, CL[l]], BF, name=f"onescol{l}")
                    nc.vector.memset(t[:], 0.0)
                    nc.vector.memset(t[:, HID:HID + 1], 1.0)
                    _ocr[l] = t
                return _ocr[l][:]

            def xlxr_layer(l):
                C = CL[l]
                for b in range(NBLK):
                    nsl = slice(b * P, (b + 1) * P)
                    pxl = ps_mm.tile([P, C], FP, space="PSUM", name="pxl2", tag="pmm")
                    for k in range(2):
                        nc.tensor.matmul(out=pxl[:], lhsT=actT[k][:, nsl],
                                         rhs=WL_sb[l][:, k * C:(k + 1) * C],
                                         start=(k == 0), stop=False)
                    nc.tensor.matmul(out=pxl[:], lhsT=ones_row[:, :P],
                                     rhs=_ones_col_row(l), start=False, stop=True)
                    xl_st = wk.tile([P, C], BF, name="xl_st2", tag="xl_st", bufs=3)
                    nc.scalar.activation(out=xl_st[:], in_=pxl[:], func=AF.Copy)
                    nc.sync.dma_start(out=xl_loc[l][nsl, :], in_=xl_st[:])

                    pxr = ps_mm.tile([P, C], FP, space="PSUM", name="pxr2", tag="pmm")
                    for k in range(2):
                        nc.tensor.matmul(out=pxr[:], lhsT=actT[k][:, nsl],
                                         rhs=WR_sb[l][:, k * C:(k + 1) * C],
                                         start=(k == 0), stop=(k == 1))
                    nc.scalar.activation(out=XRb[:, b * 260:b * 260 + C], in_=pxr[:],
                                         func=AF.Copy)
                nc.gpsimd.collective_compute(
                    "AllGather", ALU.bypass, replica_groups=[list(range(NCORES))],
                    ins=[xl_loc[l][:]], outs=[xl_full[l][:]])

            edge_layer(1, 4)
            act_prev, act_next = act_next, act_prev
            xlxr_layer(2)
            edge_layer(2, 1)
            act_prev, act_next = act_next, act_prev
            xlxr_layer(3)
            edge_layer(3, 1)

            # ---------------- MLP head ----------------
            for j in range(NLOC // 512):
                sl = slice(j * 512, (j + 1) * 512)
                pm1 = ps_mm.tile([P, 512], FP, space="PSUM", name="pm1", tag="pmm")
                for k in range(2):
                    nc.tensor.matmul(out=pm1[:DIM, :], lhsT=Wm1_sb[:, k * DIM:(k + 1) * DIM],
                                     rhs=actT[k][:, sl], start=(k == 0), stop=False)
                nc.tensor.matmul(out=pm1[:DIM, :], lhsT=bm1_sb[:], rhs=ones_row[:],
                                 start=False, stop=True)
                nc.scalar.activation(out=m1T[:DIM, sl], in_=pm1[:DIM, :], func=AF.Relu)
            for j in range(NLOC // 512):
                sl = slice(j * 512, (j + 1) * 512)
                pm2 = ps_mm.tile([P, 512], FP, space="PSUM", name="pm2", tag="pmm")
                nc.tensor.matmul(out=pm2[:DIM, :], lhsT=Wm2_sb[:], rhs=m1T[:DIM, sl],
                                 start=True, stop=False)
                nc.tensor.matmul(out=pm2[:DIM, :], lhsT=bm2_sb[:], rhs=ones_row[:],
                                 start=False, stop=True)
                nc.scalar.activation(out=m2T[:DIM, sl], in_=pm2[:DIM, :], func=AF.Relu)
            for j in range(NLOC // 512):
                sl = slice(j * 512, (j + 1) * 512)
                py = ps_mm.tile([P, 512], FP, space="PSUM", name="py", tag="pmm")
                nc.tensor.matmul(out=py[:1, :], lhsT=Wm3_sb[:], rhs=m2T[:DIM, sl],
                                 start=True, stop=False)
                nc.tensor.matmul(out=py[:1, :], lhsT=bm3_sb[:], rhs=ones_row[:],
                                 start=False, stop=True)
                nc.scalar.activation(out=y_sb[:, sl], in_=py[:1, :], func=AF.Copy)
            nc.sync.dma_start(out=out[:], in_=y_sb[:])

    nc.compile()
    return nc


_BUILD_CACHE = {}


def _get_program(NT):
    if NT not in _BUILD_CACHE:
        _BUILD_CACHE[NT] = _build(NT)
    return _BUILD_CACHE[NT]


def kernel(**inputs) -> np.ndarray:
    global LAST_RESULTS
    ii = {k: np.asarray(v) for k, v in inputs.items()}
    assert ii["x"].shape == (N, F_IN)
    for l in (1, 2, 3):
        assert not np.any(ii[f"b{l}"]), "GAT bias assumed zero"

    src_col, dst_col, NT = _prep_edges(np.asarray(ii["edge_index"], np.int64))
    w = _prep_weights(ii)
    iota = np.arange(P, dtype=BF_NP)
    iota_f = np.tile(iota[None, :], (P, 1))

    def bf(a):
        return np.asarray(a, np.float32).astype(BF_NP)

    common = dict(
        Win=bf(ii["Win"]), b_in=bf(ii["b_in"])[None, :],
        Wskip=bf(ii["Wskip"]), bskip=bf(ii["bskip"])[None, :],
        WL1=w["WL1"], WR1=w["WR1"], svec1=w["svec1"],
        WL2=w["WL2"], WR2=w["WR2"], svec2=w["svec2"],
        WL3=w["WL3"], WR3=w["WR3"], svec3=w["svec3"],
        Wm1=bf(ii["Wm1"]), bm1=bf(ii["bm1"])[None, :],
        Wm2=bf(ii["Wm2"]), bm2=bf(ii["bm2"])[None, :],
        Wm3=bf(ii["Wm3"]), bm3=bf(ii["bm3"])[None, :],
        iota_f=iota_f,
    )
    x = np.asarray(ii["x"], np.float32)
    in_maps = []
    for c in range(NCORES):
        m = dict(common)
        m["xT"] = np.ascontiguousarray(x[c * NLOC:(c + 1) * NLOC].T).astype(BF_NP)
        m["src_col"] = src_col[c]
        m["dst_col"] = dst_col[c]
        in_maps.append(m)

    nc = _get_program(NT)
    res = run_bass_kernel_spmd(nc, in_maps, list(range(NCORES)),
                               trace=bool(os.environ.get("GAT_TRACE")))
    LAST_RESULTS = res
    return np.concatenate([res.results[c]["out"].reshape(-1) for c in range(NCORES)])
